# revision 77
# baseline (speedup 1.0000x reference)
"""KNN-Attention Trainium2 kernel (8-core SPMD, batch+sequence sharded).

Full inputs in, full output out. Sharding: 8 cores = 4 batches x 2 sequence
halves. Each core receives its batch's q rotated so its own 1024 rows come
first (rows 1024:2048 are the sibling half, needed only for the kNN counts),
plus that batch's mem_table and the replicated weights.

Algorithm per core (validated against the reference on HW):
  1. qp^T = (q @ w_q)^T via PE-transposed q tiles        (d on partitions)
  2. kNN scores S = qp @ mem_table^T per 128-row l-tile; row max via DVE;
     indicator (S >= rowmax); counts accumulated with a ones-vector matmul
     per 2-l-tile batch, drained into an SBUF accumulator by DVE adds.
     Replaces argmax+gather: attention over the 1000 memory slots with
     multiplicity weights c_u is exactly attention over the 2048 gathered
     keys.
  3. K^T = (mem_table @ w_kv[:, :64])^T computed directly; V1c[u] =
     c_u * [V_u | 1] so the ones-column yields the softmax denominator and
     c_u folds in multiplicatively (no ln / no max-subtraction needed:
     |scores/8| < 3 for this input distribution).
  4. Per head (single-head pipeline): S2^T(u,l) = K^T.T @ qh^T, P =
     exp(S2/8), out'^T accumulated over u into two 1-bank PSUM halves.
     Normalize: recip of the ones-row, PE-broadcast, DVE multiply.
  5. final = out_norm @ w_concat accumulated over the 8 head-pairs.

All matmul operands are float32r (PE streams 1 row/cycle vs fp32's 4 when
the moving free dim >= 256); producers round on write per the walrus
verifier's fp32r contract. PSUM is split into a 4-slot 1-bank ring ("b1")
and a 2-slot 2-bank ring ("big2") so score tiles, transposes, s2 tiles and
the per-head output accumulators never fight for the same slots.
"""

import sys

sys.path.insert(0, "/opt/trn_rl_repo")

import numpy as np

B, L, D, N_MEM, H, DH = 4, 2048, 1024, 1000, 16, 64
LO = L // 2  # rows owned per core
NU, U = 8, 125  # u-tiles over n_mem
KT = D // 128  # 8 contraction tiles
NCH = ((0, 512), (512, 488))  # n_mem free-dim chunks, PSUM-bank aligned

_CACHED = {}


def _build_nc():
    from concourse import bacc, mybir
    import concourse.tile as tile

    F32 = mybir.dt.float32
    nc = bacc.Bacc(
        "TRN2",
        target_bir_lowering=False,
        debug=False,
        enable_asserts=False,
        num_devices=8,
    )
    q_d = nc.dram_tensor("q", [LO, D], F32, kind="ExternalInput")
    mem_d = nc.dram_tensor("mem_table", [N_MEM, D], F32, kind="ExternalInput")
    wq_d = nc.dram_tensor("w_q", [D, D], F32, kind="ExternalInput")
    wkv_d = nc.dram_tensor("w_kv", [D, 2 * DH], F32, kind="ExternalInput")
    wc_d = nc.dram_tensor("w_concat", [D, D], F32, kind="ExternalInput")
    out_d = nc.dram_tensor("out", [LO, D], F32, kind="ExternalOutput")

    with tile.TileContext(nc) as tc:
        _emit(nc, tc, q_d, mem_d, wq_d, wkv_d, wc_d, out_d)
    nc.compile()
    return nc


def _emit(nc, tc, q_d, mem_d, wq_d, wkv_d, wc_d, out_d):
    from concourse import mybir
    from concourse.masks import make_identity
    from contextlib import ExitStack

    F32 = mybir.dt.float32
    F32R = mybir.dt.float32r
    BF16 = mybir.dt.bfloat16
    AX = mybir.AxisListType
    OP = mybir.AluOpType
    ACT = mybir.ActivationFunctionType

    ctx = ExitStack()
    with ctx:
        sb = ctx.enter_context(tc.tile_pool(name="sb", bufs=1))
        ps = ctx.enter_context(tc.tile_pool(name="ps", bufs=1, space="PSUM"))
        dr = ctx.enter_context(tc.tile_pool(name="dr", bufs=1, space="DRAM"))

        ident_f = sb.tile([128, 128], F32, name="ident_f")
        make_identity(nc, ident_f)
        ident = sb.tile([128, 128], F32R, name="ident")
        nc.vector.tensor_copy(ident, ident_f)
        ones_f = sb.tile([128, 64], F32, name="ones_f")
        nc.vector.memset(ones_f, 1.0)
        ones = sb.tile([128, 64], F32R, name="ones")
        nc.vector.tensor_copy(ones, ones_f)

        qpT_own = sb.tile([128, KT, LO], F32R, name="qpT_own")
        cnt_acc = sb.tile([1, N_MEM], F32, name="cnt_acc", tag="cnta", bufs=2)
        nc.vector.memset(cnt_acc, 0.0)

        knn_calls = [0]

        def knn_ltile(lt, lhs_tile, lhs_off):
            """scores + rowmax + indicator + counts for one 128-row l-tile.

            Counts accumulate in PSUM across adjacent call pairs (b1 ring
            slots are only held for the two back-to-back calls), then DVE
            adds drain them into cnt_acc.
            """
            seq = knn_calls[0]
            knn_calls[0] += 1
            s_ps = ps.tile([128, N_MEM], F32, name=f"s_{lt}", tag="big2", bufs=2)
            for o, w in NCH:
                for k in range(KT):
                    nc.tensor.matmul(
                        s_ps[:, o : o + w],
                        lhsT=lhs_tile[:, k, lhs_off : lhs_off + 128],
                        rhs=mT[:, k, o : o + w],
                        start=(k == 0),
                        stop=(k == KT - 1),
                    )
            mx = sb.tile([128, 1], F32, name=f"mx_{lt}", tag="mx", bufs=2)
            nc.vector.reduce_max(out=mx, in_=s_ps, axis=AX.X)
            ind = sb.tile([128, N_MEM], F32R, name=f"ind_{lt}", tag="sc4", bufs=2)
            nc.vector.tensor_single_scalar(ind, s_ps, mx, OP.is_ge)
            if seq % 2 == 0:
                knn_calls.append(
                    [
                        ps.tile([1, w], F32, name=f"cnt_{lt}_{o}", tag="b1", bufs=4)
                        for o, w in NCH
                    ]
                )
            cnt_chunks = knn_calls[-1]
            for ci, (o, w) in enumerate(NCH):
                nc.tensor.matmul(
                    cnt_chunks[ci],
                    lhsT=ones[:, 0:1],
                    rhs=ind[:, o : o + w],
                    start=(seq % 2 == 0),
                    stop=(seq % 2 == 1),
                    skip_group_check=True,
                )
            if seq % 2 == 1:
                for ci, (o, w) in enumerate(NCH):
                    nc.vector.tensor_add(
                        cnt_acc[:, o : o + w], cnt_acc[:, o : o + w], cnt_chunks[ci]
                    )

        # ---- Phase 1: transpose q, qp^T = (q @ w_q)^T (own half only) ----
        # q tiles and wq stream first on the serial DMA bus (this is the
        # startup-critical data); the sibling core computes the other half's
        # counts. wq_sb is dead after the last qp matmul; pairT8 (phase 5)
        # reuses its 32KB via the shared single-slot tag. The wq DMA is split
        # per m-chunk so the first qp matmul only waits for its own slice.
        wq_sb = sb.tile([128, KT, D], F32R, name="wq_sb", tag="w32", bufs=1)
        wq_r = wq_d.ap().rearrange("(k p) m -> p k m", p=128).bitcast(F32R)
        wkv_sb = sb.tile([128, KT, 2 * DH], F32R, name="wkv_sb")
        for g in range(4):
            qT_g = sb.tile([128, KT, 256], F32R, name=f"qT_{g}", tag="qtg", bufs=2)
            for j in range(2):
                lt = 2 * g + j
                qn = sb.tile([128, D], F32R, name=f"qn_{lt}", tag="qn", bufs=3)
                nc.sync.dma_start(
                    out=qn, in_=q_d.ap()[lt * 128 : (lt + 1) * 128, :].bitcast(F32R)
                )
                trp = ps.tile([128, D], F32R, name=f"trp_{lt}", tag="big2", bufs=2)
                for k in range(KT):
                    nc.tensor.transpose(
                        trp[:, k * 128 : (k + 1) * 128],
                        qn[:, k * 128 : (k + 1) * 128],
                        ident,
                    )
                nc.vector.tensor_copy(
                    qT_g[:, :, j * 128 : (j + 1) * 128],
                    trp.rearrange("p (k c) -> p k c", k=KT),
                )
            if g == 0:
                for m in range(KT):
                    nc.sync.dma_start(
                        out=wq_sb[:, :, m * 128 : (m + 1) * 128],
                        in_=wq_r[:, :, m * 128 : (m + 1) * 128],
                    )
                nc.sync.dma_start(
                    out=wkv_sb,
                    in_=wkv_d.ap().rearrange("(k p) m -> p k m", p=128).bitcast(F32R),
                )
            for m in range(KT):
                qp_ps = ps.tile([128, 256], F32, name=f"qp_{g}_{m}", tag="b1", bufs=4)
                for k in range(KT):
                    nc.tensor.matmul(
                        qp_ps,
                        lhsT=wq_sb[:, k, m * 128 : (m + 1) * 128],
                        rhs=qT_g[:, k, :],
                        start=(k == 0),
                        stop=(k == KT - 1),
                    )
                nc.scalar.copy(qpT_own[:, m, 256 * g : 256 * g + 256], qp_ps)

        # ---- Phase 1.5: transpose mem_table -> mT (d on partitions) ----
        # 125-partition F32R transposes fail walrus codegen, so these stay
        # plain fp32; the copy into mT rounds.
        mT = sb.tile([128, KT, N_MEM], F32R, name="mT")
        for u in range(NU):
            mn = sb.tile([128, D], F32, name=f"mn_{u}", tag="qn", bufs=3)
            nc.sync.dma_start(out=mn[:U, :], in_=mem_d.ap()[u * U : (u + 1) * U, :])
            # 128-aligned k-slots so each 125-wide transpose stays in one bank
            t2 = ps.tile([128, D], F32, name=f"t2_{u}", tag="big2", bufs=2)
            for k in range(KT):
                nc.tensor.transpose(
                    t2[:, k * 128 : k * 128 + U],
                    mn[:U, k * 128 : (k + 1) * 128],
                    ident_f[:U, :U],
                )
            nc.vector.tensor_copy(
                mT[:, :, u * U : (u + 1) * U],
                t2.rearrange("p (k c) -> p k c", k=KT)[:, :, 0:U],
            )

        # ---- Phase 2: own-half kNN ----
        for lt in range(8):
            knn_ltile(lt, qpT_own, 128 * lt)

        # counts: own-half partial sums -> DRAM -> pair AllReduce with the
        # sibling core (the two cores of a batch each scored 1024 rows) ->
        # (125, 8) column layout (a partition redistribution has to bounce
        # through DRAM). The AllReduce has ~28us of fixed latency, so K^T/V
        # and the first s2+exp steps are emitted after it to keep the PE and
        # Act busy while it completes.
        cnt_dram = dr.tile([N_MEM], F32, name="cnt_dram")
        nc.sync.dma_start(out=cnt_dram.rearrange("(a b) -> a b", a=1), in_=cnt_acc)
        nc.gpsimd.collective_compute(
            "AllReduce",
            OP.add,
            replica_groups=[[0, 1], [2, 3], [4, 5], [6, 7]],
            ins=[cnt_dram.opt()],
            outs=[cnt_dram.opt()],
        )

        # w_concat is only read in phase 5b; chunked so one 4MB transfer
        # can't block the bus, and gated behind phase 2 (the WAW memset
        # keeps the otherwise dependency-free DMAs from being scheduled into
        # the startup-critical q/mem stream — they fill the AllReduce's
        # dead time instead).
        wc_sb = sb.tile([128, KT, D], F32R, name="wc_sb")
        nc.vector.tensor_copy(wc_sb[0:1, 0:1, 0:1], cnt_acc[0:1, 0:1])
        wc_r = wc_d.ap().rearrange("(k p) m -> p k m", p=128).bitcast(F32R)
        for m in range(KT):
            nc.sync.dma_start(
                out=wc_sb[:, :, m * 128 : (m + 1) * 128],
                in_=wc_r[:, :, m * 128 : (m + 1) * 128],
            )

        # ---- Phase 4a: K^T (doubled for row-packing) and V (counts-free) --
        kT2 = sb.tile([128, N_MEM], F32R, name="kT2")
        for ci, (o, w) in enumerate(NCH):
            kt_ps = ps.tile([64, w], F32, name=f"kt_{ci}", tag="b1", bufs=4)
            for k in range(KT):
                nc.tensor.matmul(
                    kt_ps,
                    lhsT=wkv_sb[:, k, 0:DH],
                    rhs=mT[:, k, o : o + w],
                    start=(k == 0),
                    stop=(k == KT - 1),
                )
            nc.vector.tensor_copy(kT2[0:64, o : o + w], kt_ps)
            nc.vector.tensor_copy(kT2[64:128, o : o + w], kt_ps)

        v_sb = sb.tile([128, NU, DH], F32, name="v_sb")
        for u in range(NU):
            v_ps = ps.tile([U, DH], F32, name=f"v_{u}", tag="b1", bufs=4)
            for k in range(KT):
                nc.tensor.matmul(
                    v_ps,
                    lhsT=mT[:, k, u * U : (u + 1) * U],
                    rhs=wkv_sb[:, k, DH : 2 * DH],
                    start=(k == 0),
                    stop=(k == KT - 1),
                )
            nc.vector.tensor_copy(v_sb[:U, u, :], v_ps)

        cnt_col = sb.tile([128, NU], F32, name="cnt_col")
        for t in range(NU):
            nc.sync.dma_start(
                out=cnt_col[:U, t : t + 1],
                in_=cnt_dram[t * U : (t + 1) * U].rearrange("(p a) -> p a", a=1),
            )

        # ---- Phase 4b: V1c[u] = c_u * [V_u | 1] ----
        v1c = sb.tile([128, NU, DH + 1], BF16, name="v1c")
        for u in range(NU):
            nc.scalar.mul(v1c[:U, u, 0:DH], v_sb[:U, u, :], mul=cnt_col[:U, u : u + 1])
            nc.vector.tensor_copy(v1c[:U, u, DH : DH + 1], cnt_col[:U, u : u + 1])

        # ---- Phase 5: attention, one head at a time ----
        # s2 tiles rotate in the 2-slot big2 ring; the per-head output
        # accumulates in two 1-bank b1 halves so consecutive heads double-
        # buffer.
        pairT8 = sb.tile([128, KT, LO], F32R, name="pairT8", tag="w32", bufs=1)
        den_dram = dr.tile([H, LO], F32, name="den_dram")

        # head-pair selector for the denominator broadcast: e8h[h', c, j] =
        # ident[h', 2c + (j >= 64)], a broadcast-AP view of the identity
        e8h = sb.tile([8, 4, 128], F32R, name="e8h", tag="sc4", bufs=2)
        for c in range(4):
            nc.vector.tensor_copy(
                e8h[:, c, :].rearrange("p (s o) -> p s o", s=2),
                ident_f[0:8, 2 * c : 2 * c + 2].broadcast_to([8, 2, 64]),
            )

        def norm_pairs(p0, n):
            # One reciprocal over the 2n denominator rows (heads on
            # partitions), then per pair a selector matmul fans 1/den out to
            # the pair's 128 head rows for one in-place multiply. Early
            # batches run while the later pairs are still accumulating; only
            # the last pair's batch sits in the tail.
            den_h = sb.tile([8, LO], F32, name=f"den_{p0}", tag="cnta", bufs=2)
            nc.sync.dma_start(
                out=den_h[0 : 2 * n, :], in_=den_dram[2 * p0 : 2 * p0 + 2 * n, :]
            )
            rden = sb.tile([8, LO], F32R, name=f"rden_{p0}", tag="cnta", bufs=2)
            with nc.allow_low_precision(reason="fp32r feeds the PE broadcast"):
                nc.vector.reciprocal(rden[0 : 2 * n, :], den_h[0 : 2 * n, :])
            for pp in range(p0, p0 + n):
                rbc = ps.tile([128, LO], F32, name=f"rbc_{pp}", tag="big2", bufs=2)
                for c2 in range(2):
                    nc.tensor.matmul(
                        rbc[:, c2 * 512 : (c2 + 1) * 512],
                        lhsT=e8h[0 : 2 * n, pp - p0, :],
                        rhs=rden[0 : 2 * n, c2 * 512 : (c2 + 1) * 512],
                        start=True,
                        stop=True,
                    )
                nc.vector.tensor_mul(
                    pairT8[:, pp, :], pairT8[:, pp, :].bitcast(F32), rbc
                )

        pending_drain = []
        for p in range(8):
            # The two heads of a pair interleave at u-step granularity so the
            # Act engine always has an independent exp queued while the PE
            # feeds the sibling stream; their four 1-bank output halves fill
            # the whole b1 ring for the duration of the pair. The previous
            # pair's normalize/drain is emitted after u=0's s2+exp so the PE
            # streams straight into this pair while the DVE drains the old
            # accumulators — no Act bubble at the boundary.
            def s2_exp(u, sub):
                h, hr = 2 * p + sub, sub * 64
                s2 = ps.tile([U, LO], F32, name=f"s2_{h}_{u}", tag="big2", bufs=2)
                for c2 in range(2):
                    nc.tensor.matmul(
                        s2[:, c2 * 512 : (c2 + 1) * 512],
                        lhsT=kT2[hr : hr + 64, u * U : (u + 1) * U],
                        rhs=qpT_own[hr : hr + 64, p, c2 * 512 : (c2 + 1) * 512],
                        start=True,
                        stop=True,
                        tile_position=(hr, 0),
                    )
                PT = sb.tile([128, LO], BF16, name=f"PT_{h}_{u}", tag="ptu", bufs=10)
                nc.scalar.activation(PT[:U, :], s2, ACT.Exp, scale=0.125)
                return PT

            def pv(u, sub, PT):
                for c2 in range(2):
                    nc.tensor.matmul(
                        o_c2[sub][c2],
                        lhsT=v1c[:U, u, :],
                        rhs=PT[:U, c2 * 512 : (c2 + 1) * 512],
                        start=(u == 0),
                        stop=(u == NU - 1),
                        skip_group_check=True,
                    )

            # 3-deep exp pipeline: three s2+exp steps are in flight before the
            # first PV, so the PE keeps feeding the Act engine while this
            # pair's first PVs wait out the previous pair's accumulator
            # drain.
            steps = [(u, sub) for u in range(NU) for sub in range(2)]
            fifo = [(u, sub, s2_exp(u, sub)) for u, sub in steps[:3]]
            for fn in pending_drain:
                fn()
            pending_drain = []
            if p == 6:
                # pairs 0-3 normalize while pairs 6-7 accumulate (their
                # drains flushed by now)
                norm_pairs(0, 4)
            o_c2 = [
                [
                    ps.tile([DH + 1, 512], F32, name=f"o_{p}_{sub}_{c2}", tag="b1", bufs=4)
                    for c2 in range(2)
                ]
                for sub in range(2)
            ]
            for u, sub in steps[3:]:
                uu, ss, PT = fifo.pop(0)
                pv(uu, ss, PT)
                fifo.append((u, sub, s2_exp(u, sub)))
            for uu, ss, PT in fifo:
                pv(uu, ss, PT)
            def make_drain(pp, oo):
                def drain():
                    # Copy the unnormalized out^T straight into pairT8 and
                    # ship the denominator rows to DRAM; division happens
                    # once, batched, after the last pair (DVE reciprocal is
                    # ~6 cycles/elem on HW, so 16K sequential elems on one
                    # partition would be ~100us — batched on 16 partitions
                    # it is ~1/16th of that).
                    for sub in range(2):
                        h, hr = 2 * pp + sub, sub * 64
                        # den staging rides the dead phase-2 ind ring (DMA
                        # cannot read PSUM directly)
                        den_sb = sb.tile([1, LO], F32, name=f"dsb_{h}", tag="sc4", bufs=2)
                        for c2 in range(2):
                            nc.vector.tensor_copy(
                                pairT8[hr : hr + 64, pp, c2 * 512 : (c2 + 1) * 512],
                                oo[sub][c2][0:DH, :],
                            )
                            nc.vector.tensor_copy(
                                den_sb[:, c2 * 512 : (c2 + 1) * 512],
                                oo[sub][c2][DH : DH + 1, :],
                            )
                        nc.sync.dma_start(
                            out=den_dram[h : h + 1, :], in_=den_sb
                        )

                return drain

            pending_drain.append(make_drain(p, o_c2))
        for fn in pending_drain:
            fn()

        norm_pairs(4, 4)

        # ---- Phase 5b: final = out_norm @ w_concat ----
        for lt in range(8):
            for c2 in range(2):
                f_ps = ps.tile([128, 512], F32, name=f"f_{lt}_{c2}", tag="b1", bufs=4)
                for p in range(8):
                    nc.tensor.matmul(
                        f_ps,
                        lhsT=pairT8[:, p, lt * 128 : (lt + 1) * 128],
                        rhs=wc_sb[:, p, c2 * 512 : (c2 + 1) * 512],
                        start=(p == 0),
                        stop=(p == 7),
                    )
                f_sb = sb.tile([128, 512], F32, name=f"fs_{lt}_{c2}", tag="qn", bufs=3)
                nc.vector.tensor_copy(f_sb, f_ps)
                nc.sync.dma_start(
                    out=out_d.ap()[
                        lt * 128 : (lt + 1) * 128, c2 * 512 : (c2 + 1) * 512
                    ],
                    in_=f_sb,
                )


def get_nc():
    if "nc" not in _CACHED:
        _CACHED["nc"] = _build_nc()
    return _CACHED["nc"]


def make_in_maps(q, mem_table, w_q, w_kv, w_concat):
    f = np.float32
    q, mem_table = np.asarray(q, f), np.asarray(mem_table, f)
    w_q, w_kv, w_concat = (
        np.ascontiguousarray(np.asarray(w_q, f)),
        np.ascontiguousarray(np.asarray(w_kv, f)),
        np.ascontiguousarray(np.asarray(w_concat, f)),
    )
    in_maps = []
    for core in range(8):
        b, half = core // 2, core % 2
        in_maps.append(
            {
                "q": np.ascontiguousarray(q[b, half * LO : (half + 1) * LO]),
                "mem_table": np.ascontiguousarray(mem_table[b]),
                "w_q": w_q,
                "w_kv": w_kv,
                "w_concat": w_concat,
            }
        )
    return in_maps


def kernel(q, kv, mem_table, w_q, w_kv, w_concat, topk, **run_kwargs):
    """Full (unsharded) inputs -> full (b, l, d) float32 output."""
    from concourse.bass_utils import run_bass_kernel_spmd

    nc = get_nc()
    in_maps = make_in_maps(q, mem_table, w_q, w_kv, w_concat)
    res = run_bass_kernel_spmd(nc, in_maps, core_ids=list(range(8)), **run_kwargs)
    out = np.zeros((B, L, D), np.float32)
    for core in range(8):
        b, half = core // 2, core % 2
        out[b, half * LO : (half + 1) * LO] = res.results[core]["out"]
    if run_kwargs:
        return out, res
    return out


# revision 84
# speedup vs baseline: 1.0087x; 1.0087x over previous
"""KNN-Attention Trainium2 kernel (8-core SPMD, batch+sequence sharded).

Full inputs in, full output out. Sharding: 8 cores = 4 batches x 2 sequence
halves. Each core receives its batch's q rotated so its own 1024 rows come
first (rows 1024:2048 are the sibling half, needed only for the kNN counts),
plus that batch's mem_table and the replicated weights.

Algorithm per core (validated against the reference on HW):
  1. qp^T = (q @ w_q)^T via PE-transposed q tiles        (d on partitions)
  2. kNN scores S = qp @ mem_table^T per 128-row l-tile; row max via DVE;
     indicator (S >= rowmax); counts accumulated with a ones-vector matmul
     per 2-l-tile batch, drained into an SBUF accumulator by DVE adds.
     Replaces argmax+gather: attention over the 1000 memory slots with
     multiplicity weights c_u is exactly attention over the 2048 gathered
     keys.
  3. K^T = (mem_table @ w_kv[:, :64])^T computed directly; V1c[u] =
     c_u * [V_u | 1] so the ones-column yields the softmax denominator and
     c_u folds in multiplicatively (no ln / no max-subtraction needed:
     |scores/8| < 3 for this input distribution).
  4. Per head (single-head pipeline): S2^T(u,l) = K^T.T @ qh^T, P =
     exp(S2/8), out'^T accumulated over u into two 1-bank PSUM halves.
     Normalize: recip of the ones-row, PE-broadcast, DVE multiply.
  5. final = out_norm @ w_concat accumulated over the 8 head-pairs.

All matmul operands are float32r (PE streams 1 row/cycle vs fp32's 4 when
the moving free dim >= 256); producers round on write per the walrus
verifier's fp32r contract. PSUM is split into a 4-slot 1-bank ring ("b1")
and a 2-slot 2-bank ring ("big2") so score tiles, transposes, s2 tiles and
the per-head output accumulators never fight for the same slots.
"""

import sys

sys.path.insert(0, "/opt/trn_rl_repo")

import numpy as np

B, L, D, N_MEM, H, DH = 4, 2048, 1024, 1000, 16, 64
LO = L // 2  # rows owned per core
NU, U = 8, 125  # u-tiles over n_mem
KT = D // 128  # 8 contraction tiles
NCH = ((0, 512), (512, 488))  # n_mem free-dim chunks, PSUM-bank aligned

_CACHED = {}


def _build_nc():
    from concourse import bacc, mybir
    import concourse.tile as tile

    F32 = mybir.dt.float32
    nc = bacc.Bacc(
        "TRN2",
        target_bir_lowering=False,
        debug=False,
        enable_asserts=False,
        num_devices=8,
    )
    q_d = nc.dram_tensor("q", [LO, D], F32, kind="ExternalInput")
    mem_d = nc.dram_tensor("mem_table", [N_MEM, D], F32, kind="ExternalInput")
    wq_d = nc.dram_tensor("w_q", [D, D], F32, kind="ExternalInput")
    wkv_d = nc.dram_tensor("w_kv", [D, 2 * DH], F32, kind="ExternalInput")
    wc_d = nc.dram_tensor("w_concat", [D, D], F32, kind="ExternalInput")
    out_d = nc.dram_tensor("out", [LO, D], F32, kind="ExternalOutput")

    with tile.TileContext(nc) as tc:
        _emit(nc, tc, q_d, mem_d, wq_d, wkv_d, wc_d, out_d)
    nc.compile()
    return nc


def _emit(nc, tc, q_d, mem_d, wq_d, wkv_d, wc_d, out_d):
    from concourse import mybir
    from concourse.masks import make_identity
    from contextlib import ExitStack

    F32 = mybir.dt.float32
    F32R = mybir.dt.float32r
    BF16 = mybir.dt.bfloat16
    AX = mybir.AxisListType
    OP = mybir.AluOpType
    ACT = mybir.ActivationFunctionType

    ctx = ExitStack()
    with ctx:
        sb = ctx.enter_context(tc.tile_pool(name="sb", bufs=1))
        ps = ctx.enter_context(tc.tile_pool(name="ps", bufs=1, space="PSUM"))
        dr = ctx.enter_context(tc.tile_pool(name="dr", bufs=1, space="DRAM"))

        ident_f = sb.tile([128, 128], F32, name="ident_f")
        make_identity(nc, ident_f)
        ident = sb.tile([128, 128], F32R, name="ident")
        nc.vector.tensor_copy(ident, ident_f)
        ones_f = sb.tile([128, 64], F32, name="ones_f")
        nc.vector.memset(ones_f, 1.0)
        ones = sb.tile([128, 64], F32R, name="ones")
        nc.vector.tensor_copy(ones, ones_f)

        qpT_own = sb.tile([128, KT, LO], F32R, name="qpT_own")
        cnt_acc = sb.tile([1, N_MEM], F32, name="cnt_acc", tag="cnta", bufs=2)
        nc.vector.memset(cnt_acc, 0.0)

        knn_calls = [0]

        def knn_ltile(lt, lhs_tile, lhs_off):
            """scores + rowmax + indicator + counts for one 128-row l-tile.

            Counts accumulate in PSUM across adjacent call pairs (b1 ring
            slots are only held for the two back-to-back calls), then DVE
            adds drain them into cnt_acc.
            """
            seq = knn_calls[0]
            knn_calls[0] += 1
            s_ps = ps.tile([128, N_MEM], F32, name=f"s_{lt}", tag="big2", bufs=2)
            for o, w in NCH:
                for k in range(KT):
                    nc.tensor.matmul(
                        s_ps[:, o : o + w],
                        lhsT=lhs_tile[:, k, lhs_off : lhs_off + 128],
                        rhs=mT[:, k, o : o + w],
                        start=(k == 0),
                        stop=(k == KT - 1),
                    )
            mx = sb.tile([128, 1], F32, name=f"mx_{lt}", tag="mx", bufs=2)
            nc.vector.reduce_max(out=mx, in_=s_ps, axis=AX.X)
            ind = sb.tile([128, N_MEM], F32R, name=f"ind_{lt}", tag="sc4", bufs=2)
            nc.vector.tensor_single_scalar(ind, s_ps, mx, OP.is_ge)
            if seq % 2 == 0:
                knn_calls.append(
                    [
                        ps.tile([1, w], F32, name=f"cnt_{lt}_{o}", tag="b1", bufs=4)
                        for o, w in NCH
                    ]
                )
            cnt_chunks = knn_calls[-1]
            for ci, (o, w) in enumerate(NCH):
                nc.tensor.matmul(
                    cnt_chunks[ci],
                    lhsT=ones[:, 0:1],
                    rhs=ind[:, o : o + w],
                    start=(seq % 2 == 0),
                    stop=(seq % 2 == 1),
                    skip_group_check=True,
                )
            if seq % 2 == 1:
                for ci, (o, w) in enumerate(NCH):
                    nc.vector.tensor_add(
                        cnt_acc[:, o : o + w], cnt_acc[:, o : o + w], cnt_chunks[ci]
                    )

        # ---- Phase 1: transpose q, qp^T = (q @ w_q)^T (own half only) ----
        # q tiles and wq stream first on the serial DMA bus (this is the
        # startup-critical data); the sibling core computes the other half's
        # counts. wq_sb is dead after the last qp matmul; pairT8 (phase 5)
        # reuses its 32KB via the shared single-slot tag. The wq DMA is split
        # per m-chunk so the first qp matmul only waits for its own slice.
        wq_sb = sb.tile([128, KT, D], F32R, name="wq_sb", tag="w32", bufs=1)
        wq_r = wq_d.ap().rearrange("(k p) m -> p k m", p=128).bitcast(F32R)
        wkv_sb = sb.tile([128, KT, 2 * DH], F32R, name="wkv_sb")
        # mem transposes stay plain fp32 (125-partition F32R transposes fail
        # walrus codegen); the copy into mT rounds.
        mT = sb.tile([128, KT, N_MEM], F32R, name="mT")

        def qp_half(g, qT_g, ms):
            for m in ms:
                qp_ps = ps.tile([128, 256], F32, name=f"qp_{g}_{m}", tag="b1", bufs=4)
                for k in range(KT):
                    nc.tensor.matmul(
                        qp_ps,
                        lhsT=wq_sb[:, k, m * 128 : (m + 1) * 128],
                        rhs=qT_g[:, k, :],
                        start=(k == 0),
                        stop=(k == KT - 1),
                    )
                nc.scalar.copy(qpT_own[:, m, 256 * g : 256 * g + 256], qp_ps)

        # qp for the upper four m-chunks is deferred one group so the PE has
        # matmul work queued before the next q tile's DMA lands, and the
        # second half of wq streams behind the first qp work.
        pending_qp = None
        for g in range(4):
            qT_g = sb.tile([128, KT, 256], F32R, name=f"qT_{g}", tag="qtg", bufs=2)
            for j in range(2):
                lt = 2 * g + j
                qn = sb.tile([128, D], F32R, name=f"qn_{lt}", tag="qn", bufs=3)
                nc.sync.dma_start(
                    out=qn, in_=q_d.ap()[lt * 128 : (lt + 1) * 128, :].bitcast(F32R)
                )
                trp = ps.tile([128, D], F32R, name=f"trp_{lt}", tag="big2", bufs=2)
                for k in range(KT):
                    nc.tensor.transpose(
                        trp[:, k * 128 : (k + 1) * 128],
                        qn[:, k * 128 : (k + 1) * 128],
                        ident,
                    )
                nc.vector.tensor_copy(
                    qT_g[:, :, j * 128 : (j + 1) * 128],
                    trp.rearrange("p (k c) -> p k c", k=KT),
                )
            if g == 0:
                for m in range(4):
                    nc.sync.dma_start(
                        out=wq_sb[:, :, m * 128 : (m + 1) * 128],
                        in_=wq_r[:, :, m * 128 : (m + 1) * 128],
                    )
                nc.sync.dma_start(
                    out=wkv_sb,
                    in_=wkv_d.ap().rearrange("(k p) m -> p k m", p=128).bitcast(F32R),
                )
            # two mem-table u-tiles per group: mT completes with the last qp
            # so the kNN phase (and the AllReduce behind it) starts earlier
            for u in (2 * g, 2 * g + 1):
                mn = sb.tile([128, D], F32, name=f"mn_{u}", tag="qn", bufs=3)
                nc.sync.dma_start(
                    out=mn[:U, :], in_=mem_d.ap()[u * U : (u + 1) * U, :]
                )
                t2 = ps.tile([128, D], F32, name=f"t2_{u}", tag="big2", bufs=2)
                for k in range(KT):
                    nc.tensor.transpose(
                        t2[:, k * 128 : k * 128 + U],
                        mn[:U, k * 128 : (k + 1) * 128],
                        ident_f[:U, :U],
                    )
                nc.vector.tensor_copy(
                    mT[:, :, u * U : (u + 1) * U],
                    t2.rearrange("p (k c) -> p k c", k=KT)[:, :, 0:U],
                )
            if pending_qp is not None:
                pending_qp()
            qp_half(g, qT_g, range(4))
            if g == 0:
                for m in range(4, KT):
                    nc.sync.dma_start(
                        out=wq_sb[:, :, m * 128 : (m + 1) * 128],
                        in_=wq_r[:, :, m * 128 : (m + 1) * 128],
                    )
            pending_qp = (lambda gg, qt: lambda: qp_half(gg, qt, range(4, KT)))(
                g, qT_g
            )
        pending_qp()

        # ---- Phase 2: own-half kNN ----
        for lt in range(8):
            knn_ltile(lt, qpT_own, 128 * lt)

        # counts: own-half partial sums -> DRAM -> pair AllReduce with the
        # sibling core (the two cores of a batch each scored 1024 rows) ->
        # (125, 8) column layout (a partition redistribution has to bounce
        # through DRAM). The AllReduce has ~28us of fixed latency, so K^T/V
        # and the first s2+exp steps are emitted after it to keep the PE and
        # Act busy while it completes.
        cnt_dram = dr.tile([N_MEM], F32, name="cnt_dram")
        nc.sync.dma_start(out=cnt_dram.rearrange("(a b) -> a b", a=1), in_=cnt_acc)
        nc.gpsimd.collective_compute(
            "AllReduce",
            OP.add,
            replica_groups=[[0, 1], [2, 3], [4, 5], [6, 7]],
            ins=[cnt_dram.opt()],
            outs=[cnt_dram.opt()],
        )

        # w_concat is only read in phase 5b; chunked so one 4MB transfer
        # can't block the bus, and gated behind phase 2 (the WAW memset
        # keeps the otherwise dependency-free DMAs from being scheduled into
        # the startup-critical q/mem stream — they fill the AllReduce's
        # dead time instead).
        wc_sb = sb.tile([128, KT, D], F32R, name="wc_sb")
        nc.vector.tensor_copy(wc_sb[0:1, 0:1, 0:1], cnt_acc[0:1, 0:1])
        wc_r = wc_d.ap().rearrange("(k p) m -> p k m", p=128).bitcast(F32R)
        for m in range(KT):
            nc.sync.dma_start(
                out=wc_sb[:, :, m * 128 : (m + 1) * 128],
                in_=wc_r[:, :, m * 128 : (m + 1) * 128],
            )

        # ---- Phase 4a: K^T (doubled for row-packing) and V (counts-free) --
        kT2 = sb.tile([128, N_MEM], F32R, name="kT2")
        for ci, (o, w) in enumerate(NCH):
            kt_ps = ps.tile([64, w], F32, name=f"kt_{ci}", tag="b1", bufs=4)
            for k in range(KT):
                nc.tensor.matmul(
                    kt_ps,
                    lhsT=wkv_sb[:, k, 0:DH],
                    rhs=mT[:, k, o : o + w],
                    start=(k == 0),
                    stop=(k == KT - 1),
                )
            nc.vector.tensor_copy(kT2[0:64, o : o + w], kt_ps)
            nc.vector.tensor_copy(kT2[64:128, o : o + w], kt_ps)

        v_sb = sb.tile([128, NU, DH], F32, name="v_sb")
        for u in range(NU):
            v_ps = ps.tile([U, DH], F32, name=f"v_{u}", tag="b1", bufs=4)
            for k in range(KT):
                nc.tensor.matmul(
                    v_ps,
                    lhsT=mT[:, k, u * U : (u + 1) * U],
                    rhs=wkv_sb[:, k, DH : 2 * DH],
                    start=(k == 0),
                    stop=(k == KT - 1),
                )
            nc.vector.tensor_copy(v_sb[:U, u, :], v_ps)

        cnt_col = sb.tile([128, NU], F32, name="cnt_col")
        for t in range(NU):
            nc.sync.dma_start(
                out=cnt_col[:U, t : t + 1],
                in_=cnt_dram[t * U : (t + 1) * U].rearrange("(p a) -> p a", a=1),
            )

        # ---- Phase 4b: V1c[u] = c_u * [V_u | 1] ----
        v1c = sb.tile([128, NU, DH + 1], BF16, name="v1c")
        for u in range(NU):
            nc.scalar.mul(v1c[:U, u, 0:DH], v_sb[:U, u, :], mul=cnt_col[:U, u : u + 1])
            nc.vector.tensor_copy(v1c[:U, u, DH : DH + 1], cnt_col[:U, u : u + 1])

        # ---- Phase 5: attention, one head at a time ----
        # s2 tiles rotate in the 2-slot big2 ring; the per-head output
        # accumulates in two 1-bank b1 halves so consecutive heads double-
        # buffer.
        pairT8 = sb.tile([128, KT, LO], F32R, name="pairT8", tag="w32", bufs=1)
        den_dram = dr.tile([H, LO], F32, name="den_dram")

        # head-pair selector for the denominator broadcast: e8h[h', c, j] =
        # ident[h', 2c + (j >= 64)], a broadcast-AP view of the identity
        e8h = sb.tile([8, 4, 128], F32R, name="e8h", tag="sc4", bufs=2)
        for c in range(4):
            nc.vector.tensor_copy(
                e8h[:, c, :].rearrange("p (s o) -> p s o", s=2),
                ident_f[0:8, 2 * c : 2 * c + 2].broadcast_to([8, 2, 64]),
            )

        def norm_pairs(p0, n):
            # One reciprocal over the 2n denominator rows (heads on
            # partitions), then per pair a selector matmul fans 1/den out to
            # the pair's 128 head rows for one in-place multiply. Early
            # batches run while the later pairs are still accumulating; only
            # the last pair's batch sits in the tail.
            den_h = sb.tile([8, LO], F32, name=f"den_{p0}", tag="cnta", bufs=2)
            nc.sync.dma_start(
                out=den_h[0 : 2 * n, :], in_=den_dram[2 * p0 : 2 * p0 + 2 * n, :]
            )
            rden = sb.tile([8, LO], F32R, name=f"rden_{p0}", tag="cnta", bufs=2)
            with nc.allow_low_precision(reason="fp32r feeds the PE broadcast"):
                nc.vector.reciprocal(rden[0 : 2 * n, :], den_h[0 : 2 * n, :])
            for pp in range(p0, p0 + n):
                rbc = ps.tile([128, LO], F32, name=f"rbc_{pp}", tag="big2", bufs=2)
                for c2 in range(2):
                    nc.tensor.matmul(
                        rbc[:, c2 * 512 : (c2 + 1) * 512],
                        lhsT=e8h[0 : 2 * n, pp - p0, :],
                        rhs=rden[0 : 2 * n, c2 * 512 : (c2 + 1) * 512],
                        start=True,
                        stop=True,
                    )
                nc.vector.tensor_mul(
                    pairT8[:, pp, :], pairT8[:, pp, :].bitcast(F32), rbc
                )

        pending_drain = []
        for p in range(8):
            # The two heads of a pair interleave at u-step granularity so the
            # Act engine always has an independent exp queued while the PE
            # feeds the sibling stream; their four 1-bank output halves fill
            # the whole b1 ring for the duration of the pair. The previous
            # pair's normalize/drain is emitted after u=0's s2+exp so the PE
            # streams straight into this pair while the DVE drains the old
            # accumulators — no Act bubble at the boundary.
            def s2_exp(u, sub):
                h, hr = 2 * p + sub, sub * 64
                s2 = ps.tile([U, LO], F32, name=f"s2_{h}_{u}", tag="big2", bufs=2)
                for c2 in range(2):
                    nc.tensor.matmul(
                        s2[:, c2 * 512 : (c2 + 1) * 512],
                        lhsT=kT2[hr : hr + 64, u * U : (u + 1) * U],
                        rhs=qpT_own[hr : hr + 64, p, c2 * 512 : (c2 + 1) * 512],
                        start=True,
                        stop=True,
                        tile_position=(hr, 0),
                    )
                PT = sb.tile([128, LO], BF16, name=f"PT_{h}_{u}", tag="ptu", bufs=12)
                nc.scalar.activation(PT[:U, :], s2, ACT.Exp, scale=0.125)
                return PT

            def pv(u, sub, PT):
                for c2 in range(2):
                    nc.tensor.matmul(
                        o_c2[sub][c2],
                        lhsT=v1c[:U, u, :],
                        rhs=PT[:U, c2 * 512 : (c2 + 1) * 512],
                        start=(u == 0),
                        stop=(u == NU - 1),
                        skip_group_check=True,
                    )

            # 3-deep exp pipeline: three s2+exp steps are in flight before the
            # first PV, so the PE keeps feeding the Act engine while this
            # pair's first PVs wait out the previous pair's accumulator
            # drain.
            steps = [(u, sub) for u in range(NU) for sub in range(2)]
            # pair 0 prefills the whole PT ring: its s2+exp stream is the
            # only count-independent work that can hide the AllReduce wait,
            # and the in-order PE queue would otherwise block at the first PV
            depth = 12 if p == 0 else 3
            fifo = [(u, sub, s2_exp(u, sub)) for u, sub in steps[:depth]]
            for fn in pending_drain:
                fn()
            pending_drain = []
            if p == 5:
                # early pairs normalize while the rest accumulate (their
                # drains flushed by now); split into small batches to limit
                # s2-ring contention
                norm_pairs(0, 2)
            elif p == 6:
                norm_pairs(2, 2)
            o_c2 = [
                [
                    ps.tile([DH + 1, 512], F32, name=f"o_{p}_{sub}_{c2}", tag="b1", bufs=4)
                    for c2 in range(2)
                ]
                for sub in range(2)
            ]
            for u, sub in steps[depth:]:
                uu, ss, PT = fifo.pop(0)
                pv(uu, ss, PT)
                fifo.append((u, sub, s2_exp(u, sub)))
            for uu, ss, PT in fifo:
                pv(uu, ss, PT)
            def make_drain(pp, oo):
                def drain():
                    # Copy the unnormalized out^T straight into pairT8 and
                    # ship the denominator rows to DRAM; division happens
                    # once, batched, after the last pair (DVE reciprocal is
                    # ~6 cycles/elem on HW, so 16K sequential elems on one
                    # partition would be ~100us — batched on 16 partitions
                    # it is ~1/16th of that).
                    for sub in range(2):
                        h, hr = 2 * pp + sub, sub * 64
                        # den staging rides the dead phase-2 ind ring (DMA
                        # cannot read PSUM directly)
                        den_sb = sb.tile([1, LO], F32, name=f"dsb_{h}", tag="sc4", bufs=2)
                        for c2 in range(2):
                            nc.vector.tensor_copy(
                                pairT8[hr : hr + 64, pp, c2 * 512 : (c2 + 1) * 512],
                                oo[sub][c2][0:DH, :],
                            )
                            nc.vector.tensor_copy(
                                den_sb[:, c2 * 512 : (c2 + 1) * 512],
                                oo[sub][c2][DH : DH + 1, :],
                            )
                        nc.sync.dma_start(
                            out=den_dram[h : h + 1, :], in_=den_sb
                        )

                return drain

            pending_drain.append(make_drain(p, o_c2))
        for fn in pending_drain:
            fn()

        norm_pairs(4, 4)

        # ---- Phase 5b: final = out_norm @ w_concat ----
        for lt in range(8):
            for c2 in range(2):
                f_ps = ps.tile([128, 512], F32, name=f"f_{lt}_{c2}", tag="b1", bufs=4)
                for p in range(8):
                    nc.tensor.matmul(
                        f_ps,
                        lhsT=pairT8[:, p, lt * 128 : (lt + 1) * 128],
                        rhs=wc_sb[:, p, c2 * 512 : (c2 + 1) * 512],
                        start=(p == 0),
                        stop=(p == 7),
                    )
                f_sb = sb.tile([128, 512], F32, name=f"fs_{lt}_{c2}", tag="qn", bufs=3)
                nc.vector.tensor_copy(f_sb, f_ps)
                nc.sync.dma_start(
                    out=out_d.ap()[
                        lt * 128 : (lt + 1) * 128, c2 * 512 : (c2 + 1) * 512
                    ],
                    in_=f_sb,
                )


def get_nc():
    if "nc" not in _CACHED:
        _CACHED["nc"] = _build_nc()
    return _CACHED["nc"]


def make_in_maps(q, mem_table, w_q, w_kv, w_concat):
    f = np.float32
    q, mem_table = np.asarray(q, f), np.asarray(mem_table, f)
    w_q, w_kv, w_concat = (
        np.ascontiguousarray(np.asarray(w_q, f)),
        np.ascontiguousarray(np.asarray(w_kv, f)),
        np.ascontiguousarray(np.asarray(w_concat, f)),
    )
    in_maps = []
    for core in range(8):
        b, half = core // 2, core % 2
        in_maps.append(
            {
                "q": np.ascontiguousarray(q[b, half * LO : (half + 1) * LO]),
                "mem_table": np.ascontiguousarray(mem_table[b]),
                "w_q": w_q,
                "w_kv": w_kv,
                "w_concat": w_concat,
            }
        )
    return in_maps


def kernel(q, kv, mem_table, w_q, w_kv, w_concat, topk, **run_kwargs):
    """Full (unsharded) inputs -> full (b, l, d) float32 output."""
    from concourse.bass_utils import run_bass_kernel_spmd

    nc = get_nc()
    in_maps = make_in_maps(q, mem_table, w_q, w_kv, w_concat)
    res = run_bass_kernel_spmd(nc, in_maps, core_ids=list(range(8)), **run_kwargs)
    out = np.zeros((B, L, D), np.float32)
    for core in range(8):
        b, half = core // 2, core % 2
        out[b, half * LO : (half + 1) * LO] = res.results[core]["out"]
    if run_kwargs:
        return out, res
    return out
